# revision 18
# baseline (speedup 1.0000x reference)
# Trainium2 Bass kernel for CoAttentionModule (axial co-attention, 8 heads).
#
# Sharding: data-parallel over (direction, batch) = 2 x 4 = 8 NeuronCores.
# Core c computes weighted = _coattention(qf, rf)[b].T for its (d, b) pair;
# the host concatenates [features, weighted] per direction.
#
# On-chip layout: the hw axis is w-major everywhere (col = w*32 + i, i = h
# index); the host pre-permutes features and un-permutes the output. This
# makes every matmul stationary operand a contiguous SBUF slice (walrus
# requires single-free-dim weight APs).
#
# Precision plan (rel err ~1.7e-2 vs the 2e-2 gate, numpy-validated):
#   Q/K projections: single-pass fp8 DoubleRow (e4m3 weights AND e4m3
#     activations, both pre-scaled on host: x*16, W*1024 so weight values
#     clear the e4m3 subnormal region). The resulting q/k are stored bf16 at
#     16384x their true scale; the 16384^-2 is folded into the softmax exp
#     scale, and rel-embedding constants are host-scaled by 16384 to match.
#   V projection: fp8 DoubleRow, x-side hi+lo (e4m3 + e5m2 residual, near
#     exact) x single e4m3 Wv, x-stationary over the full 2048 outs with 4
#     MMs per LDWEIGHTS; consecutive w-group chains double-buffer by
#     borrowing the (idle during the prepass) scores/AV/QAUG PSUM banks.
#   O projection: plain bf16 (streaming floor), attention output bf16.
# Measured rel err 0.0190 vs the 2e-2 gate.
#
# Per-core pipeline (fp32 PSUM accumulation everywhere):
#   qT = Wq8.T @ xq8 (+bq*S)      [c_out, hw]  fp8 DR single pass
#   kT = Wk8.T @ xr8  + RWF*S     RWF[c,(w,k)] = rel_emb[(k-w)%63, c]  (rel_w
#                                 folded into keys; bk cancels in softmax)
#   v  = (xh+xl).T @ Wv8          [(w,k), c]   fp8 DR, 16384x scale in sv2
#   QAUG[t', col(w,i)] = sum_c relx[(t'-i)%63, c] q[c, col]  (only rows
#                                 t'<32 matter: kaug one-hot needs t'==k<32)
#   scores tile (head n, w-group of 4) [128=(w,i), 128=(w,k)]:
#       q.k' + QAUG.KAUG(one-hot) + WIND.KMASK(-1e30 off-diag mask channels)
#   softmax: exp(scale=1/(16*16384^2)) with accum_out row sums -> reciprocal
#   probsT via DVE 32x32 stream transpose (block-diagonal => exact transpose)
#   avT[c,(w,i)] = v.T @ probsT (bf16); outT = Wob.T @ attT + bo'  bf16
#   (bv folded on host: bo' = bv @ Wo + bo; bk dropped: softmax-invariant)
import numpy as np
import ml_dtypes

B, C, H, W = 4, 2048, 32, 32
HW = H * W
NH, HD = 8, 256
T = 2 * max(H, W) - 1  # 63
NC = C // 128  # 16 chunks
SX = 16.0      # activation fp8 pre-scale
SW = 1024.0    # weight fp8 pre-scale
SQ = SX * SW   # scale of stored q/k relative to true values

_CACHE = {}


def _hostprep(Wq, bq, Wk, bk, Wv, bv, Wo, bo, rel_emb):
    bf = ml_dtypes.bfloat16
    f8 = ml_dtypes.float8_e4m3
    f32 = np.float32
    Wq, Wk, Wv, Wo = (np.asarray(a, f32) for a in (Wq, Wk, Wv, Wo))
    rel = np.asarray(rel_emb, f32)  # [63, 256]
    ii = np.arange(32)

    # lhsT blobs [co, p, ci*128+m]: one contiguous [128, 2048] DMA per co chunk
    def lchunks(Wm):
        return np.ascontiguousarray(
            Wm.reshape(NC, 128, NC, 128).transpose(2, 1, 0, 3).reshape(NC, 128, C))


    def swpack(blob):  # [NC,128,C] -> DoubleRowSwInterleave layout per cj pair
        b = blob.reshape(NC, 128, NC // 2, 2, 128)  # [co,p,cj,ab,m]
        out = np.empty_like(b)
        out[:, :, :, 0, :] = b[:, :, :, 0, ::-1]
        out[:, :, :, 1, :] = b[:, :, :, 1, ::-1]
        # interleave per logical column: stored col 2j = A[127-j], 2j+1 = B[127-j]
        return np.ascontiguousarray(
            out.transpose(0, 1, 2, 4, 3).reshape(NC, 128, C))

    wq8 = swpack(lchunks(Wq * SW)).astype(f8)
    wk8 = swpack(lchunks(Wk * SW)).astype(f8)
    wob = lchunks(Wo).astype(bf)
    # V weights as moving blob [p, ci*2048 + m] = SW*Wv[ci*128+p, m], e4m3
    wv8 = np.ascontiguousarray(
        (Wv * SW).reshape(NC, 128, C).transpose(1, 0, 2).reshape(128, NC * C)
    ).astype(f8)

    bq_c = np.ascontiguousarray((np.asarray(bq, f32) * SQ).reshape(NC, 128).T)
    bo2 = np.asarray(bv, f32) @ Wo + np.asarray(bo, f32)
    bo2_c = np.ascontiguousarray(bo2.reshape(NC, 128).T)  # [128,16]

    w_idx, k_idx = np.meshgrid(np.arange(32), np.arange(32), indexing="ij")
    # rel_w fold table, w-major [2, 128, 1024], scaled to stored-k units:
    # rwf[ch, p, w*32+k] = SQ * rel[(k-w)%63, ch*128+p]
    rwf = rel[(k_idx - w_idx) % T].reshape(HW, HD) * SQ  # [(w,k), 256]
    rwf = np.ascontiguousarray(rwf.T.reshape(2, 128, HW)).astype(bf)
    # two-period rel_emb.T for QAUG, scaled by SQ so QAUG = q_s * relx is at
    # SQ^2 like q_s*k_s: relx[p, ch*126+u] = SQ * rel[u%63, ch*128+p]
    relx = np.empty((128, 2 * 2 * T), f32)
    for ch in range(2):
        blk = rel[np.arange(2 * T) % T, ch * 128:(ch + 1) * 128]  # [126,128]
        relx[:, ch * 2 * T:(ch + 1) * 2 * T] = blk.T * SQ
    relx = relx.astype(bf)
    # key-side aug channels [96, 1024] w-major: rows 0:63 one-hot rel gather
    # (kaug[t, w*32+k] = t==k), row 63 zero, rows 64:96 block-diag mask
    # (kmask[w', w*32+k] = 0 if w==w' else -1e30). Query side: rows 0:63 QAUG,
    # row 63 zero, rows 64:96 w-indicator.
    kaug = np.zeros((96, HW), f32)
    kaug[k_idx.reshape(-1), np.arange(HW)] = 1.0
    kaug[64:96] = -1e30
    wind = np.zeros((32, HW), f32)
    for w in range(32):
        wind[w, w * 32 + ii] = 1.0  # query col w*32+i
        kaug[64 + w, w * 32 + ii] = 0.0  # key col w*32+k
    kaug = kaug.astype(bf)
    wind = wind.astype(bf)

    return dict(wq8=wq8, wk8=wk8, wob=wob, wv8=wv8, bq_c=bq_c, bo2_c=bo2_c,
                rwf=rwf, relx=relx, kaug=kaug, wind=wind)


def _build(timing_twin=False, loop=1):
    import concourse.bacc as bacc
    import concourse.mybir as mybir
    import concourse.tile as tile

    F32, BF16 = mybir.dt.float32, mybir.dt.bfloat16
    F8 = mybir.dt.float8e4
    F5 = mybir.dt.float8e5
    DR = mybir.MatmulPerfMode.DoubleRow
    DRS = mybir.MatmulPerfMode.DoubleRowSwInterleave
    nc = bacc.Bacc(None, target_bir_lowering=False)

    if timing_twin:
        # timing-equivalent NEFF: big tensors live in internal DRAM scratch
        # (no per-call host staging), only a tiny external in/out pair.
        def declare(name, shape, dt, isOutput=False):
            return nc.dram_tensor(name, shape, dt)
        tiny_in = nc.declare_dram_parameter("tiny_in", [1, 4], F32, isOutput=False)
        tiny_out = nc.declare_dram_parameter("tiny_out", [1, 4], F32, isOutput=True)
    else:
        declare = nc.declare_dram_parameter

    xq = declare("xq", [C, HW], F8, isOutput=False)
    xr8 = declare("xr8", [C, HW], F8, isOutput=False)
    xrl = declare("xrl", [C, HW], F5, isOutput=False)
    wq8 = declare("wq8", [NC, 128, C], F8, isOutput=False)
    wk8 = declare("wk8", [NC, 128, C], F8, isOutput=False)
    wob = declare("wob", [NC, 128, C], BF16, isOutput=False)
    wv8 = declare("wv8", [128, NC * C], F8, isOutput=False)
    bq_c = declare("bq_c", [128, NC], F32, isOutput=False)
    bo2_c = declare("bo2_c", [128, NC], F32, isOutput=False)
    rwf = declare("rwf", [2, 128, HW], BF16, isOutput=False)
    relx = declare("relx", [128, 2 * 2 * T], BF16, isOutput=False)
    kaug = declare("kaug", [96, HW], BF16, isOutput=False)
    wind = declare("wind", [32, HW], BF16, isOutput=False)
    out = declare("out", [C, HW], F32, isOutput=True)

    EXP = mybir.ActivationFunctionType.Exp
    ACOPY = mybir.ActivationFunctionType.Copy
    ESCALE = 1.0 / (16.0 * SQ * SQ)

    with tile.TileContext(nc) as tc:
        with (
            tc.tile_pool(name="feat", bufs=3) as feat_pool,
            tc.tile_pool(name="featb", bufs=1) as featb_pool,
            tc.tile_pool(name="const", bufs=1) as const_pool,
            tc.tile_pool(name="head", bufs=3) as head_pool,
            tc.tile_pool(name="vfull", bufs=1) as vfull_pool,
            tc.tile_pool(name="wstr8", bufs=3) as wstr8_pool,
            tc.tile_pool(name="wstrb", bufs=3) as wstrb_pool,
            tc.tile_pool(name="probs", bufs=2) as probs_pool,
            tc.tile_pool(name="outs", bufs=2) as outs_pool,
            tc.tile_pool(name="psum", bufs=4, space="PSUM") as psum_pool,
            tc.tile_pool(name="psumb", bufs=2, space="PSUM") as psumb_pool,
            tc.tile_pool(name="psumav", bufs=1, space="PSUM") as psumav_pool,
            tc.tile_pool(name="psumq", bufs=1, space="PSUM") as psumq_pool,
        ):
            # ---- load features + constants (resident) ----
            # xq first (gates the very first Q-proj groups), then xr8/xrb,
            # then constants so the PE cold-start wait is minimal.
            xqt = feat_pool.tile([128, NC * HW], F8, tag="feat8")
            xr8t = feat_pool.tile([128, NC * HW], F8, tag="feat8")
            xrlt = feat_pool.tile([128, NC * HW], F5, tag="feat8")
            attb = featb_pool.tile([128, NC * HW], BF16, tag="featb")
            for cc in range(NC):
                nc.sync.dma_start(xqt[:, cc * HW:(cc + 1) * HW], xq[cc * 128:(cc + 1) * 128, :])
            for cc in range(NC):
                nc.sync.dma_start(xr8t[:, cc * HW:(cc + 1) * HW], xr8[cc * 128:(cc + 1) * 128, :])
            for cc in range(NC):
                nc.sync.dma_start(xrlt[:, cc * HW:(cc + 1) * HW], xrl[cc * 128:(cc + 1) * 128, :])
            c_wv = const_pool.tile([128, NC * C], F8)
            nc.sync.dma_start(c_wv[:], wv8[:])

            c_kaug = const_pool.tile([96, HW], BF16)
            nc.sync.dma_start(c_kaug[:], kaug[:])
            c_wind = const_pool.tile([32, HW], BF16)
            nc.sync.dma_start(c_wind[:], wind[:])
            c_rwf = const_pool.tile([128, 2 * HW], BF16)
            nc.sync.dma_start(c_rwf[:, 0:HW], rwf[0])
            nc.sync.dma_start(c_rwf[:, HW:2 * HW], rwf[1])
            c_relx = const_pool.tile([128, 2 * 2 * T], BF16)
            nc.sync.dma_start(c_relx[:], relx[:])
            c_bq = const_pool.tile([128, NC], F32)
            nc.sync.dma_start(c_bq[:], bq_c[:])
            c_bo = const_pool.tile([128, NC], F32)
            nc.sync.dma_start(c_bo[:], bo2_c[:])

            x3q = xqt[:, :].rearrange("p (ci hw) -> p ci hw", ci=NC)
            x3r = xr8t[:, :].rearrange("p (ci hw) -> p ci hw", ci=NC)
            x3rl = xrlt[:, :].rearrange("p (ci hw) -> p ci hw", ci=NC)
            wv3 = c_wv[:, :].rearrange("p (ci m) -> p ci m", ci=NC)
            a3 = attb[:, :].rearrange("p (cc hw) -> p cc hw", cc=NC)

            def att_scores(n, sq, sk, sqa, prts):
                # ---- stage 1: scores + softmax chain for head n (issued one
                # head behind the projections). prts collects probsT tiles.
                for wg in range(8):
                    sct = psumb_pool.tile([128, 128], F32, tag="sa", name="sct")
                    sc = sct[:]
                    nc.tensor.matmul(sc, sq[:, wg * 128:(wg + 1) * 128],
                                     sk[:, wg * 128:(wg + 1) * 128],
                                     start=True, stop=False)
                    nc.tensor.matmul(sc, sq[:, HW + wg * 128: HW + (wg + 1) * 128],
                                     sk[:, HW + wg * 128: HW + (wg + 1) * 128],
                                     start=False, stop=False)
                    nc.tensor.matmul(sc, sqa[:, wg * 128:(wg + 1) * 128],
                                     c_kaug[:, wg * 128:(wg + 1) * 128],
                                     start=False, stop=True)
                    probs = probs_pool.tile([128, 128], BF16, tag="pr")
                    sums = probs_pool.tile([128, 1], F32, tag="sm")
                    recip = probs_pool.tile([128, 1], F32, tag="rc")
                    nc.scalar.activation(probs[:], sc, EXP, scale=ESCALE,
                                         accum_out=sums[:])
                    nc.vector.reciprocal(recip[:], sums[:])
                    nc.any.tensor_scalar_mul(probs[:], probs[:], recip[:])
                    probsT = probs_pool.tile([128, 128], BF16, tag="prT")
                    nc.vector.transpose(probsT[:], probs[:])
                    prts.append(probsT)

            def att_av(n, sv2, prts):
                # ---- stage 2: attention-weighted V for head n (issued two
                # heads behind; all probsT inputs are long done -> no PE waits)
                for wg in range(8):
                    av = pav[:, (wg % 2) * 256:(wg % 2 + 1) * 256]
                    for ch in range(2):
                        svbase = wg * C + n * HD + ch * 128
                        nc.tensor.matmul(
                            av[:, ch * 128:(ch + 1) * 128],
                            sv2[:, svbase: svbase + 128],
                            prts[wg][:], start=True, stop=True)
                    nc.scalar.activation(
                        a3[:, n * 2:n * 2 + 2, wg * 128:(wg + 1) * 128],
                        av.rearrange("p (ch x) -> p ch x", ch=2), ACOPY,
                        scale=1.0 / SQ)

            for rep in range(loop):
                stage1 = None  # (n, sq, sk, sqa, prts)
                stage2 = None  # (n, sv2, prts)
                for n in range(NH):
                    sq = head_pool.tile([128, 2 * HW], BF16, tag="sq")
                    sk = head_pool.tile([128, 2 * HW], BF16, tag="sk")
                    sqa = head_pool.tile([96, HW], BF16, tag="sqa")
                    if n == 0:
                        # ---- V projection, all heads in one prepass: fp8 DR,
                        # x-stationary hi(e4m3)+lo(e5m2) residual pair, Wv8
                        # moving over the full 2048 outs (4 banks, 4 MMs/LDW).
                        # sv2 holds 16384*v (scales undone in the AV copy). ----
                        sv2 = vfull_pool.tile([128, 8 * C], BF16, tag="sv2")
                        for wg in range(8):
                            # alternate between the projection banks and the
                            # (idle at this point) scores/AV/QAUG banks so
                            # consecutive w-group chains double-buffer.
                            if wg % 2 == 0:
                                pss4 = [psum_pool.tile([128, 512], F32, tag="pp",
                                                       name=f"psv{pp}")
                                        for pp in range(4)]
                            else:
                                pb0 = psumb_pool.tile([128, 512], F32, tag="sa", name="pb0")
                                pb1 = psumb_pool.tile([128, 512], F32, tag="sa", name="pb1")
                                pb2 = psumav_pool.tile([128, 512], F32, tag="av", name="pb2")
                                pb3 = psumq_pool.tile([128, 512], F32, tag="qa", name="pb3")
                                pss4 = [pb0, pb1, pb2, pb3]
                            for cj in range(NC // 2):
                                for t, x3t in ((0, x3r), (1, x3rl)):
                                    for pp in range(4):
                                        nc.tensor.matmul(
                                            pss4[pp][:],
                                            x3t[:, 2 * cj:2 * cj + 2,
                                                wg * 128:(wg + 1) * 128],
                                            wv3[:, 2 * cj:2 * cj + 2,
                                                pp * 512:(pp + 1) * 512],
                                            start=(cj == 0 and t == 0),
                                            stop=(cj == NC // 2 - 1 and t == 1),
                                            perf_mode=DR)
                            for pp in range(4):
                                nc.any.tensor_copy(
                                    sv2[:, wg * C + pp * 512: wg * C + (pp + 1) * 512],
                                    pss4[pp][:])
                        # AV accumulator bank, re-allocated per rep so the V
                        # prepass above can borrow this pool's bank.
                        pav = psumav_pool.tile([128, 512], F32, tag="av")
                    # aug rows 32:64 zero (kaug one-hot rows t>=32 are all
                    # zero, so sqa rows 32:63 never contribute; keep finite);
                    # rows 64:96 w-indicator. head_pool rotates over 3
                    # buffers, and rows 32:96 are never overwritten, so only
                    # the first three heads (one init per buffer) need this.
                    if rep == 0 and n < 3:
                        nc.vector.memset(sqa[32:64, :], 0.0)
                        nc.vector.tensor_copy(sqa[64:96, :], c_wind[:])

                    # ---- Q / K projections: W.T @ x, single-pass scaled fp8
                    # DoubleRow chains into one PSUM ----
                    for which in range(2):  # 0 = Q, 1 = K
                        hsrc = wq8 if which == 0 else wk8
                        x3 = x3q if which == 0 else x3r
                        dst = sq if which == 0 else sk
                        for co2 in range(2):
                            co = n * 2 + co2
                            wt_h = wstr8_pool.tile([128, C], F8, tag="wl8")
                            nc.sync.dma_start(wt_h[:], hsrc[co])
                            w3h = wt_h[:, :].rearrange("p (ci m) -> p ci m", ci=NC)
                            pss = [psum_pool.tile([128, 512], F32, tag="pp",
                                                  name=f"psqk{h2}")
                                   for h2 in range(2)]
                            for cj in range(NC // 2):
                                for h2 in range(2):
                                    nc.tensor.matmul(
                                        pss[h2][:],
                                        wt_h[:, cj * 256:(cj + 1) * 256],
                                        x3[:, 2 * cj:2 * cj + 2, h2 * 512:(h2 + 1) * 512],
                                        start=(cj == 0),
                                        stop=(cj == NC // 2 - 1),
                                        perf_mode=DRS)
                            for h2 in range(2):
                                dpos = dst[:, co2 * HW + h2 * 512: co2 * HW + (h2 + 1) * 512]
                                if which == 0:
                                    nc.any.tensor_scalar_add(dpos, pss[h2][:], c_bq[:, co:co + 1])
                                else:
                                    nc.any.tensor_add(
                                        dpos, pss[h2][:],
                                        c_rwf[:, co2 * HW + h2 * 512: co2 * HW + (h2 + 1) * 512])

                    # ---- QAUG: per query-row i, rolled rel_emb.T contraction.
                    # Only out rows t'<32 matter (kaug one-hot needs t'==k,
                    # k<32), so the stationary is the 32-col slice
                    # relx[:, 63-i : 95-i] (cheap LDWEIGHTS). Four i's run
                    # concurrently via PE column tiling (tile_position
                    # (0, 32j)): out partitions 32j:32j+32 hold i = ig*4+j.
                    pqa = psumq_pool.tile([128, 256], F32, tag="qa")
                    for ig in range(8):
                        for j in range(4):
                            i = ig * 4 + j
                            for ch in range(2):
                                nc.tensor.matmul(
                                    pqa[32 * j:32 * j + 32, ig * 32:(ig + 1) * 32],
                                    c_relx[:, ch * 2 * T + T - i: ch * 2 * T + T + 32 - i],
                                    sq[:, ch * HW + i: (ch + 1) * HW: 32],
                                    start=(ch == 0), stop=(ch == 1),
                                    tile_position=(0, 32 * j))
                    # pqa[32j+t', ig*32+w] = QAUG[t', col(w, ig*4+j)]
                    for j in range(4):
                        nc.any.tensor_copy(
                            sqa[0:32, :].rearrange("p (w ig j) -> p j ig w", ig=8, j=4)[:, j, :, :],
                            pqa[32 * j:32 * j + 32, :].rearrange("p (ig w) -> p ig w", w=32))

                    # pipelined attention: scores of head n-1, AV of n-2.
                    if stage1 is not None:
                        s1n, s1q, s1k, s1a, s1v = stage1
                        prts = []
                        att_scores(s1n, s1q, s1k, s1a, prts)
                        if stage2 is not None:
                            att_av(*stage2)
                        stage2 = (s1n, s1v, prts)
                    stage1 = (n, sq, sk, sqa, sv2)
                s1n, s1q, s1k, s1a, s1v = stage1
                prts = []
                att_scores(s1n, s1q, s1k, s1a, prts)
                att_av(*stage2)

                # ---- output projection, bf16 weight-stationary. The first
                # two co chunks run their cj 0..13 partial chains BEFORE
                # att_av(7): head 7's att channels (cj 14,15) are only needed
                # at the chain tails, so the PE stays busy while head 7's
                # softmax chain finishes on DVE/ACT. ----
                def o_chunk_open(co):
                    wt_b = wstrb_pool.tile([128, C], BF16, tag="wlb", name="wt_b")
                    nc.sync.dma_start(wt_b[:], wob[co])
                    w3b = wt_b[:, :].rearrange("p (ci m) -> p ci m", ci=NC)
                    pss = [psum_pool.tile([128, 512], F32, tag="pp",
                                          name=f"pso{h2}")
                           for h2 in range(2)]
                    for cj in range(NC - 2):
                        for h2 in range(2):
                            nc.tensor.matmul(
                                pss[h2][:], w3b[:, cj, :],
                                a3[:, cj, h2 * 512:(h2 + 1) * 512],
                                start=(cj == 0), stop=False)
                    return w3b, pss

                def o_chunk_close(co, w3b, pss):
                    for cj in range(NC - 2, NC):
                        for h2 in range(2):
                            nc.tensor.matmul(
                                pss[h2][:], w3b[:, cj, :],
                                a3[:, cj, h2 * 512:(h2 + 1) * 512],
                                start=False, stop=(cj == NC - 1))
                    for h2 in range(2):
                        ot = outs_pool.tile([128, 512], F32, tag="ot")
                        nc.any.tensor_scalar_add(ot[:], pss[h2][:], c_bo[:, co:co + 1])
                        nc.sync.dma_start(
                            out[co * 128:(co + 1) * 128, h2 * 512:(h2 + 1) * 512], ot[:])

                open0 = o_chunk_open(0)
                open1 = o_chunk_open(1)
                att_av(s1n, s1v, prts)
                o_chunk_close(0, *open0)
                o_chunk_close(1, *open1)
                for co in range(2, NC):
                    w3b, pss = o_chunk_open(co)
                    o_chunk_close(co, w3b, pss)

                if timing_twin:
                    tt = outs_pool.tile([1, 4], F32, tag="tt")
                    nc.sync.dma_start(tt[:], tiny_in[:])
                    nc.sync.dma_start(tiny_out[:], tt[:])

            if timing_twin:
                tt = outs_pool.tile([1, 4], F32, tag="tt")
                nc.sync.dma_start(tt[:], tiny_in[:])
                nc.sync.dma_start(tiny_out[:], tt[:])

    nc.finalize()
    return nc


def kernel(left_features, right_features, Wq, bq, Wk, bk, Wv, bv, Wo, bo, rel_emb,
           _trace=False):
    from concourse.bass_utils import run_bass_kernel_spmd

    if "nc" not in _CACHE:
        _CACHE["nc"] = _build()
    nc = _CACHE["nc"]

    consts = _hostprep(Wq, bq, Wk, bk, Wv, bv, Wo, bo, rel_emb)
    lf = np.asarray(left_features, np.float32)
    rf = np.asarray(right_features, np.float32)

    f8 = ml_dtypes.float8_e4m3
    f5 = ml_dtypes.float8_e5m2

    def wmajor(x):  # (C, H, W) -> (C, HW) with col = w*32 + i
        return np.ascontiguousarray(x.transpose(0, 2, 1).reshape(C, HW))

    in_maps = []
    for core in range(8):
        d, b = divmod(core, 4)
        qf = lf[b] if d == 0 else rf[b]
        rfb = rf[b] if d == 0 else lf[b]
        m = dict(consts)
        wq_ = wmajor(qf)
        wr_ = wmajor(rfb)
        xr_hi = (wr_ * SX).astype(f8)
        m["xq"] = (wq_ * SX).astype(f8)
        m["xr8"] = xr_hi
        m["xrl"] = (wr_ * SX - xr_hi.astype(np.float32)).astype(f5)
        in_maps.append(m)

    res = run_bass_kernel_spmd(nc, in_maps, list(range(8)), trace=_trace)
    _CACHE["last_result"] = res

    def unperm(o):  # [C, HW w-major] -> (C, H, W)
        return np.ascontiguousarray(o.reshape(C, W, H).transpose(0, 2, 1))

    wr = np.stack([unperm(res.results[b]["out"]) for b in range(4)])
    wl = np.stack([unperm(res.results[4 + b]["out"]) for b in range(4)])
    left_att = np.concatenate([lf, wr], axis=1)
    right_att = np.concatenate([rf, wl], axis=1)
    return (left_att, right_att)
